# revision 14
# baseline (speedup 1.0000x reference)
"""Leaky-integrator linear recurrence kernel for Trainium2.

u_t = TAU * u_{t-1} + x_t along the last (time) axis of x[32, 1024, 2048] f32.

Strategy: data-parallel across 8 NeuronCores (4 batches each), 16-bit HBM
traffic (the 2e-2 tolerance dwarfs bf16 quantization), and the recurrence
computed on the Tensor engine as a triangular matmul with the carry folded
into the contraction. In a host-transposed layout xt[time, rows], time is
cut into blocks of 127 steps. The moving tile of block b is

    partition 0      : u[t0-1, :]   (carry row; zeros for block 0 via a
                                     host-prepended zero row in xt)
    partitions 1..127: x[t0 .. t0+126, :]

and ONE constant stationary S[k, m] (S[0, m] = TAU^(m+1) carry column,
S[1+j, m] = TAU^(m-j) for j <= m) yields the exact block output
u[t0 .. t0+126] in a single FD=512 pass per PSUM chunk:

    u[t0+m] = TAU^(m+1) u[t0-1] + sum_j TAU^(m-j) x[t0+j]

Because every matmul shares the same stationary, all but the first
LDWEIGHTS are redundant; _dedup_ldweights() removes them (~100 ns of PE
each). This halves Tensor-engine work versus the two-matmul (cross-block
band + triangular band) formulation and takes PE off the critical path.
The carry row travels between consecutive blocks' SBUF tiles via tiny
per-chunk SWDGE DMAs (gpsimd queue — its own rings, so they never queue
behind megabyte slab/output transfers). A 16-step remainder block with its
own small stationary covers 2048 = 16*127 + 16.

Engine assignment: Sync issues input DMAs, Scalar issues output DMAs (two
HWDGE rings — input prefetch never head-of-line blocks behind output
drain), GpSimd issues the carry DMAs, Tensor does the matmuls, and the
PSUM f32 -> SBUF bf16 downcast copies alternate between Vector and Scalar.

The walrus build in this container allows at most ONE embedded sync-wait
per engine instruction (two on EventSemaphore); Tile's wait assignment can
attach several. _split_excess_waits() hoists the extras onto standalone
EventSemaphore instructions inserted immediately before, on the same
engine — conservative but correct, since every awaited semaphore's
producer precedes the waiter in the scheduled program order.
"""

import numpy as np
import ml_dtypes

import concourse.bass as bass
import concourse.mybir as mybir
from concourse.bass_utils import run_bass_kernel_spmd
from concourse.tile import TileContext

TAU = 0.9
B, F, T = 32, 1024, 2048
N_CORES = 8
B_PER_CORE = B // N_CORES          # 4
ROWS = B_PER_CORE * F              # 4096 independent recurrences per core
P = 128
BLK = P - 1                        # 127 time steps per full block
N_BLK = T // BLK                   # 16 full blocks
REM = T - N_BLK * BLK              # 16-step remainder
CHUNK = 512                        # PSUM bank width (f32)
N_CHUNK = ROWS // CHUNK            # 8

NP_DT = ml_dtypes.bfloat16
MYBIR_DT = mybir.dt.bfloat16

_nc_cache = None
_coef_cache = None
last_results = None  # BassKernelResults from the most recent run (for test.py)


def _split_excess_waits(nc: bass.Bass) -> None:
    for fn in nc.m.functions:
        for blk in fn.blocks:
            out = []
            changed = False
            for inst in blk.instructions:
                si = inst.sync_info
                waits = list(si.on_wait) if si is not None else []
                cap = 2 if inst.opcode == "EventSemaphore" else 1
                if len(waits) <= cap:
                    out.append(inst)
                    continue
                changed = True
                # On DMAs keep a queue-ordering (DMAHW*) wait embedded so
                # queue-level throttling stays at the queue; otherwise keep
                # the last wait.
                keep_idx = len(waits) - 1
                if inst.opcode == "DMACopy":
                    for k, w in enumerate(waits):
                        if (w.ant_name or "").startswith("DMA"):
                            keep_idx = k
                            break
                rest = [w for j, w in enumerate(waits) if j != keep_idx]
                for j in range(0, len(rest), 2):
                    out.append(
                        mybir.InstEventSemaphore(
                            name=f"{inst.name}-xw{j}",
                            opcode="EventSemaphore",
                            engine=inst.engine,
                            debug=inst.debug,
                            sync_info=mybir.SyncInfo(
                                on_wait=rest[j : j + 2], on_update=[]
                            ),
                        )
                    )
                inst.sync_info = mybir.SyncInfo(
                    on_wait=[waits[keep_idx]], on_update=list(si.on_update)
                )
                out.append(inst)
            if changed:
                blk.instructions = out


def _dedup_ldweights(nc: bass.Bass) -> None:
    """Drop PE weight reloads that reload the already-loaded stationary.

    tile_legalize splits every matmul into InstLdweights + a
    non-self-loading InstMatmult. Matmult does not clobber the PE weight
    array, so consecutive Ldweights with an identical weights AP are
    redundant — all but the first can go (~100 ns of PE time each). A
    redundant Ldweights that carries semaphore waits/updates is replaced
    by an EventSemaphore on the same engine so the synchronization is
    preserved; any other PE instruction resets the tracked signature.
    """
    for fn in nc.m.functions:
        for blk in fn.blocks:
            out = []
            changed = False
            last_sig = None
            for inst in blk.instructions:
                if inst.opcode == "Matmult":
                    out.append(inst)
                    continue
                if inst.opcode != "Ldweights":
                    if inst.engine == mybir.EngineType.PE and inst.opcode not in (
                        "EventSemaphore",
                    ):
                        last_sig = None
                    out.append(inst)
                    continue
                a = inst.ins[0]
                sig = (a.memref, a.offset, str(a.ap), str(a.dtype))
                if sig != last_sig:
                    last_sig = sig
                    out.append(inst)
                    continue
                changed = True
                si = inst.sync_info
                waits = list(si.on_wait) if si is not None else []
                upds = list(si.on_update) if si is not None else []
                if waits or upds:
                    for j in range(0, max(len(waits), 1), 2):
                        out.append(
                            mybir.InstEventSemaphore(
                                name=f"{inst.name}-lw{j}",
                                opcode="EventSemaphore",
                                engine=inst.engine,
                                debug=inst.debug,
                                sync_info=mybir.SyncInfo(
                                    on_wait=waits[j : j + 2],
                                    on_update=upds if j == 0 else [],
                                ),
                            )
                        )
            if changed:
                blk.instructions = out


def _coef() -> np.ndarray:
    # [P, BLK + REM] = [S | S2] packed side by side (one SBUF tile/DMA).
    # S  [128, 127]: S[0, m]  = TAU^(m+1); S[1+j, m] = TAU^(m-j) for j<=m.
    # S2 [17, 16] in rows 0..16 of the last REM columns: same structure.
    def stat(n):
        j = np.arange(n)[None, :]           # output row m
        k = np.arange(n)[:, None]           # x offset j
        tri = np.where(k <= j, TAU ** np.maximum(j - k, 0).astype(np.float64), 0.0)
        top = TAU ** (np.arange(n, dtype=np.float64) + 1.0)[None, :]
        return np.vstack([top, tri])        # [n+1, n]

    c = np.zeros((P, BLK + REM), dtype=np.float64)
    c[0 : P, 0:BLK] = stat(BLK)
    c[0 : REM + 1, BLK : BLK + REM] = stat(REM)
    return np.ascontiguousarray(c.astype(NP_DT))


def _build() -> bass.Bass:
    nc = bass.Bass()
    # xt row 0 is a host-written zero row: block 0's carry. Row 1+t = x[t].
    xt = nc.dram_tensor("xt", [T + 1, ROWS], MYBIR_DT, kind="ExternalInput")
    coef = nc.dram_tensor("coef", [P, BLK + REM], MYBIR_DT, kind="ExternalInput")
    yt = nc.dram_tensor("yt", [T, ROWS], MYBIR_DT, kind="ExternalOutput")

    with TileContext(nc) as tc:
        with (
            tc.tile_pool(name="const", bufs=1) as cpool,
            tc.tile_pool(name="in", bufs=8) as ipool,
            tc.tile_pool(name="out", bufs=4) as opool,
            tc.tile_pool(name="psum", bufs=8, space="PSUM") as ppool,
        ):
            cf = cpool.tile([P, BLK + REM], MYBIR_DT)
            nc.scalar.dma_start(out=cf[:], in_=coef[:])
            cS = cf[:, 0:BLK]                      # [128, 127]
            cS2 = cf[0 : REM + 1, BLK : BLK + REM]  # [17, 16]

            prev_ut = None
            for b in range(N_BLK + 1):
                last = b == N_BLK
                kp = (REM + 1) if last else P      # moving partitions
                om = REM if last else BLK          # output rows
                t0 = b * BLK
                s = ipool.tile([P, ROWS], MYBIR_DT)
                if b == 0:
                    # carry row 0 comes from the host zero row: one full DMA
                    nc.sync.dma_start(out=s[0:P, :], in_=xt[0:P, :])
                else:
                    nc.sync.dma_start(
                        out=s[1:kp, :], in_=xt[t0 + 1 : t0 + kp, :]
                    )
                utile = opool.tile([P, ROWS], MYBIR_DT)
                for c in range(N_CHUNK):
                    sl = slice(c * CHUNK, (c + 1) * CHUNK)
                    if b > 0:
                        # inject the carry u[t0-1] into moving partition 0
                        nc.gpsimd.dma_start(
                            out=s[0:1, sl], in_=prev_ut[BLK - 1 : BLK, sl]
                        )
                    pt = ppool.tile([P, CHUNK], mybir.dt.float32)
                    nc.tensor.matmul(
                        pt[0:om, :],
                        lhsT=cS2 if last else cS,
                        rhs=s[0:kp, sl],
                        start=True,
                        stop=True,
                    )
                    if c % 2 == 0:
                        nc.vector.tensor_copy(utile[0:om, sl], pt[0:om, :])
                    else:
                        nc.scalar.copy(utile[0:om, sl], pt[0:om, :])
                nc.scalar.dma_start(out=yt[t0 : t0 + om, :], in_=utile[0:om, :])
                prev_ut = utile

    _dedup_ldweights(nc)
    _split_excess_waits(nc)
    return nc


def kernel(x: np.ndarray, **_unused) -> np.ndarray:
    global _nc_cache, _coef_cache, last_results
    if _nc_cache is None:
        _nc_cache = _build()
        _coef_cache = _coef()
    nc = _nc_cache

    x = np.asarray(x)
    assert x.shape == (B, F, T), x.shape
    x16 = np.ascontiguousarray(x.reshape(N_CORES, ROWS, T), dtype=NP_DT)
    in_maps = []
    for c in range(N_CORES):
        xt = np.zeros((T + 1, ROWS), dtype=NP_DT)
        xt[1:] = x16[c].T
        in_maps.append({"xt": xt, "coef": _coef_cache})
    last_results = run_bass_kernel_spmd(
        nc, in_maps, core_ids=list(range(N_CORES))
    )
    out = np.concatenate(
        [
            r["yt"].T.astype(np.float32).reshape(B_PER_CORE, F, T)
            for r in last_results.results
        ],
        axis=0,
    )
    return out
